# revision 3
# baseline (speedup 1.0000x reference)
"""Trainium2 Bass kernel for a ViT attention block (LN -> QKV -> RoPE -> attn -> out-proj).

Sharding: 8 cores = 2 batches x 4 head-groups (4 heads each). Each core computes
a partial out-projection (its 4 heads) for one batch, transposed as [D, N].
Host sums the 4 partials per batch and transposes back. LayerNorm gamma/beta are
folded into the QKV weights/bias on the host; the softmax scale and log2(e) are
folded into the Q-side weights so attention logits arrive in log2 units.

Device pipeline (per core):
  - Phase A (per 128-token tile): LN stats on DVE, normalize to bf16, then a
    DMA-xbar transpose ([128,1024] -> [128,8,128]) writes xT directly -- no PE
    transposes, no PSUM->SBUF copies. Grouped in 5 token groups so QKV matmuls
    start as soon as group 0 is transposed (keeps PE warm from ~10us in).
  - Phase B (per token group): Q,K projected into [feat, tok] (lhsT = weight
    chunks), bias via K=1 matmul with a ones row that is ZERO on padded tokens.
    PSUM->SBUF copies on the scalar engine; RoPE (rotate-half via partition-shift
    DMAs) in bf16 on DVE. V projected into [tok, feat] with an appended ones
    column (softmax denominator comes for free in attn@v).
  - Attention per (head-pair, 1024-wide q-chunk): dots for the two heads of a
    pair run CONCURRENTLY on disjoint PE row halves (K=64 each, auto
    tile_position from base partition 0/64). exp runs in the log2 domain:
    ~70% of [128,1024] tiles on ACT (Exp, scale=ln2), ~30% on DVE via a
    two-sample Schraudolph (int16 bit-trick + shifted twin, added as bf16 --
    rms error 0.6%). attn@v accumulates [65, q] per head with the ones column
    giving the denominator. Normalization via reciprocal + ones-outer-product
    broadcast; out-proj chunks are interleaved into the next pair's attention
    so the PE never idles; results DMA out as [D, N] chunks as they finish.
All matmuls bf16 with f32 PSUM accumulation.
"""

import sys

sys.path.insert(0, "/opt/trn_rl_repo")

import numpy as np
import ml_dtypes

import concourse.bacc as bacc
import concourse.mybir as mybir
import concourse.tile as tile
from concourse.bass_utils import run_bass_kernel_spmd

F32 = mybir.dt.float32
BF16 = mybir.dt.bfloat16
I16 = mybir.dt.int16
AF = mybir.ActivationFunctionType
OP = mybir.AluOpType
BF = ml_dtypes.bfloat16

B, N, D = 2, 2049, 1024
DH = 64
HPC = 4  # heads per core
NT = 17  # 128-token tiles (padded to 2176)
TPAD = NT * 128
SCALE = DH ** -0.5
LOG2E = 1.4426950408889634
LN2 = 0.6931471805599453
B_SCH = 16085.88  # two-sample Schraudolph bias (calibrated, see module docstring)
# q-column chunks [offset, width]; the tail chunk is the single real token 2048
JJ = [(0, 1024), (1024, 1024)]
JTAIL = (2048, 1)
# token groups backing the 5 xT tiles (4+4+4+4+1 of the 17 token tiles)
TG = [(0, 512), (512, 512), (1024, 512), (1536, 512), (2048, 128)]


def _subs(jw):
    return [(s, min(512, jw - s)) for s in range(0, jw, 512)]


def _tg_of(col):
    return min(col // 512, 4)


def _build():
    nc = bacc.Bacc("TRN2", target_bir_lowering=False, debug=False, num_devices=8)

    x_d = nc.declare_dram_parameter("x", [N, D], F32, False)
    wqk_d = nc.declare_dram_parameter("wqk", [D, 512], BF16, False)
    wv_d = nc.declare_dram_parameter("wv", [D, 256], BF16, False)
    wo_d = nc.declare_dram_parameter("wo", [256, D], BF16, False)
    bqk_d = nc.declare_dram_parameter("bqk", [1, 512], BF16, False)
    bv_d = nc.declare_dram_parameter("bv", [1, 256], BF16, False)
    cos_d = nc.declare_dram_parameter("cos2", [128, TPAD], BF16, False)
    sin_d = nc.declare_dram_parameter("sinf2", [128, TPAD], BF16, False)
    out_d = nc.declare_dram_parameter("out", [D, N], F32, True)

    with tile.TileContext(nc) as tc:
        with (
            tc.tile_pool(name="const", bufs=1) as cpool,
            tc.tile_pool(name="persist", bufs=1) as ppool,
            tc.tile_pool(name="work", bufs=2) as wpool,
            tc.tile_pool(name="psum", bufs=2, space="PSUM") as pspool,
        ):
            # ---------------- constants ----------------
            wqk_sb = [cpool.tile([128, 512], BF16, tag=f"wqk{c}", name=f"wqk{c}") for c in range(8)]
            wv_sb = [cpool.tile([128, 256], BF16, tag=f"wv{c}", name=f"wv{c}") for c in range(8)]
            wo_sb = [cpool.tile([128, 1024], BF16, tag=f"wo{c}", name=f"wo{c}") for c in range(2)]
            bqk_sb = cpool.tile([1, 512], BF16, tag="bqk", name="bqk")
            bv_sb = cpool.tile([1, 256], BF16, tag="bv", name="bv")
            cos_sb = cpool.tile([128, TPAD], BF16, tag="cos", name="cos")
            sin_sb = cpool.tile([128, TPAD], BF16, tag="sin", name="sin")
            ones_sb = cpool.tile([1, TPAD], BF16, tag="ones", name="ones")

            def _load_weights():
                for c in range(8):
                    nc.sync.dma_start(out=wqk_sb[c][:], in_=wqk_d[c * 128:(c + 1) * 128, :])
                    nc.sync.dma_start(out=wv_sb[c][:], in_=wv_d[c * 128:(c + 1) * 128, :])

            def _load_consts():
                for c in range(2):
                    nc.gpsimd.dma_start(out=wo_sb[c][:], in_=wo_d[c * 128:(c + 1) * 128, :])
                nc.gpsimd.dma_start(out=bqk_sb[:], in_=bqk_d[:])
                nc.gpsimd.dma_start(out=bv_sb[:], in_=bv_d[:])
                nc.gpsimd.dma_start(out=cos_sb[:], in_=cos_d[:])
                nc.gpsimd.dma_start(out=sin_sb[:], in_=sin_d[:])

            # ones over real tokens only: zero on padding so bias never lands
            # on padded tokens/columns
            nc.vector.memset(ones_sb[:], 1.0)
            nc.vector.memset(ones_sb[:, N:], 0.0)
            eps_sb = cpool.tile([128, 1], F32, tag="eps", name="eps")
            nc.vector.memset(eps_sb[:], 1e-5)
            ones64_sb = cpool.tile([128, 64], BF16, tag="ones64", name="ones64")
            nc.vector.memset(ones64_sb[:], 1.0)

            # ---------------- persistent activations ----------------
            xTg = [
                ppool.tile([128, 8, tw], BF16, tag=f"xT{g}", name=f"xT{g}")
                for g, (to, tw) in enumerate(TG)
            ]

            def xslice(c, jo, jw):
                g = _tg_of(jo)
                to, tw = TG[g]
                assert jo + jw <= to + tw
                return xTg[g][:, c, jo - to:jo - to + jw]

            # qkT tiles: 0,1 = q head-pairs (h01, h23); 2,3 = k head-pairs
            qkT_sb = [ppool.tile([128, TPAD], BF16, tag=f"qkT{f}", name=f"qkT{f}") for f in range(4)]
            vaug_sb = [ppool.tile([128, 260], BF16, tag=f"v{k}", name=f"v{k}") for k in range(NT)]

            # ---------------- phase A: LayerNorm + DMA-xbar transpose ----------------
            def ln_tile(i):
                xa = wpool.tile([128, D], F32, tag="xa", name="xa", bufs=3)
                if i < 16:
                    nc.sync.dma_start(out=xa[:], in_=x_d[i * 128:(i + 1) * 128, :])
                else:
                    nc.vector.memset(xa[:], 0.0)
                    nc.sync.dma_start(out=xa[0:1, :], in_=x_d[2048:2049, :])
                if i == 1:
                    _load_weights()
                if i == 0:
                    _load_consts()
                stats = wpool.tile([128, 12], F32, tag="stats", name="stats", bufs=3)
                mv = wpool.tile([128, 2], F32, tag="mv", name="mv", bufs=4)
                nc.vector.bn_stats(stats[:, 0:6], xa[:, 0:512])
                nc.vector.bn_stats(stats[:, 6:12], xa[:, 512:1024])
                nc.vector.bn_aggr(mv[:], stats[:])
                std = wpool.tile([128, 1], F32, tag="std", name="std", bufs=4)
                rstd = wpool.tile([128, 1], F32, tag="rstd", name="rstd", bufs=4)
                murstd = wpool.tile([128, 1], F32, tag="murstd", name="murstd")
                nc.scalar.activation(std[:], mv[:, 1:2], AF.Sqrt, bias=eps_sb[:])
                nc.vector.reciprocal(rstd[:], std[:])
                nc.vector.tensor_mul(murstd[:], mv[:, 0:1], rstd[:])
                xn = wpool.tile([128, D], BF16, tag="xn", name="xn", bufs=4)
                nc.vector.tensor_scalar(
                    xn[:], xa[:], rstd[:], murstd[:], OP.mult, OP.subtract
                )
                g = _tg_of(i * 128)
                to, tw = TG[g]
                off = i * 128 - to
                nc.scalar.dma_start_transpose(
                    out=xTg[g][:, :, off:off + 128], in_=xn[:]
                )

            # ---------------- phase B: QKV projection + RoPE (per group) ----------------
            def qk_group(g):
                to, tw = TG[g]
                for f in range(4):
                    psq = pspool.tile([128, tw], F32, tag="po", name="psq", bufs=4)
                    for c in range(8):
                        nc.tensor.matmul(
                            psq[:],
                            wqk_sb[c][:, f * 128:(f + 1) * 128],
                            xTg[g][:, c, :],
                            start=(c == 0),
                            stop=False,
                        )
                    nc.tensor.matmul(
                        psq[:],
                        bqk_sb[:, f * 128:(f + 1) * 128],
                        ones_sb[:, to:to + tw],
                        start=False,
                        stop=True,
                    )
                    qf = wpool.tile([128, tw], BF16, tag="qf", name="qf", bufs=3)
                    nc.scalar.copy(qf[:], psq[:])
                    qs = wpool.tile([128, tw], BF16, tag="qs", name="qs", bufs=3)
                    nc.sync.dma_start(out=qs[0:32, :], in_=qf[32:64, :])
                    nc.sync.dma_start(out=qs[32:64, :], in_=qf[0:32, :])
                    nc.sync.dma_start(out=qs[64:96, :], in_=qf[96:128, :])
                    nc.sync.dma_start(out=qs[96:128, :], in_=qf[64:96, :])
                    t1 = wpool.tile([128, tw], BF16, tag="t1", name="t1", bufs=2)
                    t2 = wpool.tile([128, tw], BF16, tag="t2", name="t2", bufs=2)
                    nc.vector.tensor_mul(t1[:], qf[:], cos_sb[:, to:to + tw])
                    nc.vector.tensor_mul(t2[:], qs[:], sin_sb[:, to:to + tw])
                    nc.vector.tensor_add(qkT_sb[f][:, to:to + tw], t1[:], t2[:])

            def v_tile(k):
                psv = pspool.tile([128, 256], F32, tag="po", name="psv", bufs=4)
                for c in range(8):
                    nc.tensor.matmul(
                        psv[:],
                        xslice(c, k * 128, 128),
                        wv_sb[c][:],
                        start=(c == 0),
                        stop=False,
                    )
                nc.tensor.matmul(
                    psv[:],
                    ones_sb[:, k * 128:(k + 1) * 128],
                    bv_sb[:],
                    start=False,
                    stop=True,
                )
                va = vaug_sb[k]
                va3 = va[:, :].rearrange("p (a b) -> p a b", a=4)
                nc.scalar.copy(
                    va3[:, :, 0:64], psv[:, :].rearrange("p (a b) -> p a b", a=4)
                )
                if k < 16:
                    nc.vector.memset(va3[:, :, 64:65], 1.0)
                else:
                    # only token 2048 is real; zero ones-col on padded keys
                    nc.vector.memset(va3[:, :, 64:65], 0.0)
                    nc.vector.memset(va3[0:1, :, 64:65], 1.0)

            # A/B interleaved per token-group: LN tiles of group g+1 overlap
            # QKV matmuls of group g
            for g in range(5):
                t0 = TG[g][0] // 128
                t1_ = (TG[g][0] + TG[g][1]) // 128
                for i in range(t0, t1_):
                    ln_tile(i)
                qk_group(g)
                for k in range(t0, t1_):
                    v_tile(k)

            # ---------------- attention ----------------
            exp_ctr = [0]

            def emit_exp(ex_ap, psd_ap, jw):
                idx = exp_ctr[0]
                exp_ctr[0] += 1
                if jw >= 512 and (idx % 10) in (2, 5, 8):
                    i16 = wpool.tile([128, 1024], I16, tag="i16", name="i16", bufs=2)
                    j16 = wpool.tile([128, 1024], I16, tag="j16", name="j16", bufs=2)
                    nc.vector.tensor_scalar(
                        i16[:, 0:jw], psd_ap, 128.0, B_SCH, OP.mult, OP.add
                    )
                    nc.vector.tensor_scalar(j16[:, 0:jw], i16[:, 0:jw], 64, None, OP.add)
                    nc.vector.tensor_add(
                        ex_ap, i16[:, 0:jw].bitcast(BF16), j16[:, 0:jw].bitcast(BF16)
                    )
                else:
                    nc.scalar.activation(ex_ap, psd_ap, AF.Exp, scale=LN2)

            def att_pair(p, jo, jw, oev_l, dnp, side):
                qt = qkT_sb[p]
                ktile = qkT_sb[2 + p]
                pso = [
                    [
                        pspool.tile([65, 512], F32, tag="po", name=f"pso{hh}{si}", bufs=4)
                        for si in range(len(_subs(jw)))
                    ]
                    for hh in range(2)
                ]
                for k in range(NT):
                    psd = [
                        pspool.tile([128, 1024], F32, tag="dots", name=f"psd{hh}", bufs=2)
                        for hh in range(2)
                    ]
                    for so, sw in _subs(jw):
                        for hh in range(2):
                            po = hh * 64
                            nc.tensor.matmul(
                                psd[hh][:, so:so + sw],
                                ktile[po:po + 64, k * 128:(k + 1) * 128],
                                qt[po:po + 64, jo + so:jo + so + sw],
                            )
                    ex = [
                        wpool.tile([128, 1024], BF16, tag="ex", name="ex", bufs=4)
                        for _ in range(2)
                    ]
                    for hh in range(2):
                        emit_exp(ex[hh][:, 0:jw], psd[hh][:, 0:jw], jw)
                    for hh in range(2):
                        h = 2 * p + hh
                        for si, (so, sw) in enumerate(_subs(jw)):
                            nc.tensor.matmul(
                                pso[hh][si][:, 0:sw],
                                vaug_sb[k][:, h * 65:h * 65 + 65],
                                ex[hh][:, so:so + sw],
                                start=(k == 0),
                                stop=(k == NT - 1),
                                skip_group_check=True,
                            )
                    if side and k % 2 == 1:
                        side.pop(0)()
                for hh in range(2):
                    h = 2 * p + hh
                    oev = wpool.tile([65, 1024], BF16, tag="oev", name="oev", bufs=4)
                    for si, (so, sw) in enumerate(_subs(jw)):
                        if si % 2 == 0:
                            nc.vector.tensor_copy(oev[:, so:so + sw], pso[hh][si][:, 0:sw])
                        else:
                            nc.scalar.copy(oev[:, so:so + sw], pso[hh][si][:, 0:sw])
                    oev_l[h] = oev
                    nc.gpsimd.dma_start(
                        out=dnp[p][32 * hh:32 * hh + 1, 0:jw],
                        in_=oev[64:65, 0:jw],
                    )

            def att_head_tail(jo, h, oev_l, dnp):
                # single q column (token 2048): dots as 17 N=1 matmuls
                qt = qkT_sb[h // 2]
                ktile = qkT_sb[2 + h // 2]
                po = (h % 2) * 64
                pso = pspool.tile([65, 512], F32, tag="po", name="psot", bufs=4)
                psd = pspool.tile([128, 1024], F32, tag="dots", name="psdt", bufs=2)
                for k in range(NT):
                    nc.tensor.matmul(
                        psd[:, k:k + 1],
                        ktile[po:po + 64, k * 128:(k + 1) * 128],
                        qt[po:po + 64, jo:jo + 1],
                    )
                ex = wpool.tile([128, 1024], BF16, tag="ex", name="ex", bufs=4)
                nc.scalar.activation(ex[:, 0:NT], psd[:, 0:NT], AF.Exp, scale=LN2)
                for k in range(NT):
                    nc.tensor.matmul(
                        pso[:, 0:1],
                        vaug_sb[k][:, h * 65:h * 65 + 65],
                        ex[:, k:k + 1],
                        start=(k == 0),
                        stop=(k == NT - 1),
                        skip_group_check=True,
                    )
                oev = wpool.tile([65, 64], BF16, tag="oevt", name="oevt", bufs=4)
                nc.vector.tensor_copy(oev[:, 0:1], pso[:, 0:1])
                oev_l[h] = oev
                nc.gpsimd.dma_start(
                    out=dnp[h // 2][32 * (h % 2):32 * (h % 2) + 1, 0:1],
                    in_=oev[64:65, 0:1],
                )

            def epi_pair(jo, jw, p, oev_l, dnp, an):
                dnb = wpool.tile([64, 1024], BF16, tag=f"dnb{p}", name=f"dnb{p}", bufs=2)
                for so, sw in _subs(jw):
                    dnr = wpool.tile([64, 512], F32, tag="dnr", name="dnr", bufs=2)
                    nc.vector.reciprocal(dnr[:, 0:sw], dnp[p][:, so:so + sw])
                    nc.vector.tensor_copy(dnb[:, so:so + sw], dnr[:, 0:sw])
                for hh in range(2):
                    h = 2 * p + hh
                    hr = 32 * hh
                    for so, sw in _subs(jw):
                        psb = pspool.tile([64, 512], F32, tag="po", name="psb", bufs=4)
                        nc.tensor.matmul(
                            psb[:, 0:sw],
                            ones64_sb[hr:hr + 1, :],
                            dnb[hr:hr + 1, so:so + sw],
                        )
                        if hh == 0:
                            nc.vector.tensor_mul(
                                an[p][0:64, so:so + sw],
                                oev_l[h][0:64, so:so + sw],
                                psb[:, 0:sw],
                            )
                        else:
                            nt = wpool.tile([64, 512], BF16, tag="nt", name="nt", bufs=4)
                            nc.vector.tensor_mul(
                                nt[:, 0:sw], oev_l[h][0:64, so:so + sw], psb[:, 0:sw]
                            )
                            nc.gpsimd.dma_start(
                                out=an[p][64:128, so:so + sw], in_=nt[:, 0:sw]
                            )

            def emit_op_chunk(jo, jw, an, of, so, sw, final=False):
                swr = min(sw, max(0, N - (jo + so)))
                pp = pspool.tile([128, 512], F32, tag="dots", name="pp", bufs=2)
                nc.tensor.matmul(
                    pp[:, 0:sw],
                    wo_sb[0][:, of * 128:(of + 1) * 128],
                    an[0][:, so:so + sw],
                    start=True,
                    stop=False,
                )
                nc.tensor.matmul(
                    pp[:, 0:sw],
                    wo_sb[1][:, of * 128:(of + 1) * 128],
                    an[1][:, so:so + sw],
                    start=False,
                    stop=True,
                )
                oo = wpool.tile([128, 512], F32, tag="oo", name="oo", bufs=3)
                if of % 2 == 0:
                    nc.vector.tensor_copy(oo[:, 0:swr], pp[:, 0:swr])
                else:
                    nc.scalar.copy(oo[:, 0:swr], pp[:, 0:swr])
                nc.sync.dma_start(
                    out=out_d[of * 128:(of + 1) * 128, jo + so:jo + so + swr],
                    in_=oo[:, 0:swr],
                )

            side = []
            for jo, jw in JJ:
                an = [
                    wpool.tile([128, 1024], BF16, tag=f"an{ch}", name=f"an{ch}", bufs=2)
                    for ch in range(2)
                ]
                dnp = [
                    wpool.tile([64, 1024], BF16, tag=f"dnp{p}", name=f"dnp{p}", bufs=2)
                    for p in range(2)
                ]
                oev_l = [None] * 4
                att_pair(0, jo, jw, oev_l, dnp, side)
                att_pair(1, jo, jw, oev_l, dnp, side)
                epi_pair(jo, jw, 0, oev_l, dnp, an)
                epi_pair(jo, jw, 1, oev_l, dnp, an)
                while side:
                    side.pop(0)()
                side = [
                    (lambda fin=False, jo=jo, jw=jw, an=an, of=of, so=so, sw=sw:
                     emit_op_chunk(jo, jw, an, of, so, sw, fin))
                    for of in range(8)
                    for so, sw in _subs(jw)
                ]

            # tail: q column 2048
            jo, jw = JTAIL
            an = [
                wpool.tile([128, 1024], BF16, tag=f"an{ch}", name=f"an{ch}", bufs=2)
                for ch in range(2)
            ]
            dnp = [
                wpool.tile([64, 1024], BF16, tag=f"dnp{p}", name=f"dnp{p}", bufs=2)
                for p in range(2)
            ]
            oev_l = [None] * 4
            att_head_tail(jo, 0, oev_l, dnp)
            att_head_tail(jo, 1, oev_l, dnp)
            if side:
                side.pop(0)()
            epi_pair(jo, jw, 0, oev_l, dnp, an)
            att_head_tail(jo, 2, oev_l, dnp)
            att_head_tail(jo, 3, oev_l, dnp)
            if side:
                side.pop(0)()
            epi_pair(jo, jw, 1, oev_l, dnp, an)
            while side:
                side.pop(0)()
            for of in range(8):
                emit_op_chunk(jo, jw, an, of, 0, 1, True)

    nc.compile()
    return nc


_NC = None


def _get_nc():
    global _NC
    if _NC is None:
        _NC = _build()
    return _NC


def _make_inputs(x, ln_gamma, ln_beta, w_qkv, w_out):
    w_eff = (w_qkv * ln_gamma[:, None].astype(np.float32)).astype(np.float32)
    b_all = (ln_beta.astype(np.float32) @ w_qkv.astype(np.float32)).astype(np.float32)
    # fold softmax scale and log2(e) into the Q side: logits arrive in log2 units
    qfac = np.float32(SCALE * LOG2E)

    inv = 1.0 / (10000.0 ** (np.arange(0, 64, 2, dtype=np.float64) / 64.0))
    fr = np.arange(2048, dtype=np.float64)[:, None] * inv[None, :]
    cos64 = np.concatenate([np.cos(fr), np.cos(fr)], axis=1).T  # [64, 2048]
    sinf64 = np.concatenate([-np.sin(fr), np.sin(fr)], axis=1).T
    cos2 = np.ones((128, TPAD), np.float32)
    sinf2 = np.zeros((128, TPAD), np.float32)
    cos2[:, 1:2049] = np.tile(cos64, (2, 1)).astype(np.float32)
    sinf2[:, 1:2049] = np.tile(sinf64, (2, 1)).astype(np.float32)

    in_maps = []
    for c in range(8):
        b, g = c // 4, c % 4
        cols = slice(256 * g, 256 * g + 256)
        wq = w_eff[:, 0:1024][:, cols] * qfac
        wk = w_eff[:, 1024:2048][:, cols]
        wqk = np.concatenate([wq, wk], axis=1).astype(BF)
        wv = w_eff[:, 2048:3072][:, cols].astype(BF)
        wo = w_out[cols, :].astype(BF)
        bq = b_all[0:1024][cols] * qfac
        bk = b_all[1024:2048][cols]
        bqk = np.concatenate([bq, bk])[None, :].astype(BF)
        bv = b_all[2048:3072][cols][None, :].astype(BF)
        in_maps.append(
            {
                "x": np.ascontiguousarray(x[b]).astype(np.float32),
                "wqk": np.ascontiguousarray(wqk),
                "wv": np.ascontiguousarray(wv),
                "wo": np.ascontiguousarray(wo),
                "bqk": bqk,
                "bv": bv,
                "cos2": cos2.astype(BF),
                "sinf2": sinf2.astype(BF),
            }
        )
    return in_maps


def kernel(x, ln_gamma, ln_beta, w_qkv, w_out, _trace=False, _trace_kwargs=None):
    nc = _get_nc()
    in_maps = _make_inputs(x, ln_gamma, ln_beta, w_qkv, w_out)
    res = run_bass_kernel_spmd(
        nc, in_maps, core_ids=list(range(8)), trace=_trace,
        **(_trace_kwargs or {}),
    )
    out = np.zeros((B, N, D), np.float32)
    for c in range(8):
        out[c // 4] += np.asarray(res.results[c]["out"], np.float32).T
    if _trace:
        return out, res
    return out
